# revision 6
# baseline (speedup 1.0000x reference)
"""Trainium2 Bass kernel for nn_BBConv (GNN message passing).

Computation (reference):
    x = features @ weight                       # [N, DIN] @ [DIN, DOUT]
    agg = segment_sum(values * x[col], row, N)  # COO SpMM
    h = elu(agg + bias)
    out = layernorm(h) * gamma + beta           # LN over feature dim

Algebraic restructure: segment_sum commutes with the dense transform:
    agg_pre = segment_sum(values * features[col], row, N)   # [N, DIN]
    agg = agg_pre @ weight

Device strategy (8 NeuronCores, SPMD, identical instruction stream):
  - Destination nodes sharded: core c owns rows [c*12500, (c+1)*12500), padded
    to 12544 = 98 tiles of 128 rows.
  - features cast to fp16 on host, replicated to all cores' HBM as the gather
    table; edges' source rows are gathered per-edge ("slots") with
    gpsimd.dma_gather (int16 indices -> table split into banks of 32768 rows).
  - Per dest-tile t: slots grouped in blocks of 128.  For each block:
      S[slot, d] = value[slot] * (dest_local[slot] == d)   (one DVE
      tensor_scalar op vs an iota constant), then one PE matmul accumulates
      psum[feat, dest] += Xg[slot, feat].T @ S[slot, dest]  over all blocks.
  - Epilogue per tile: W-matmul (f32), bias+ELU (exact: relu(z) + min(exp(z),1)
    - 1), PE transpose back to node-major, LayerNorm on DVE/ACT, DMA out.
  - All per-core differences live in data (idx / dest-id / value arrays),
    never in the instruction stream, so one Bass program runs SPMD on 8 cores.
"""

import sys

for _p in ("/opt/trn_rl_repo", "/opt/pypackages"):
    if _p not in sys.path:
        sys.path.append(_p)

import numpy as np

import concourse.bass as bass
import concourse.bacc as bacc
import concourse.mybir as mybir
import concourse.tile as tile
from concourse import bass_utils

F16 = mybir.dt.float16
F32 = mybir.dt.float32
I16 = mybir.dt.int16
AX = mybir.AxisListType
OP = mybir.AluOpType
ACT = mybir.ActivationFunctionType

N_NODES = 100000
N_CORES = 8
DIN = 128
DOUT = 128
P = 128
BANK = 32768
EPS = 1e-5
_DST_BUFS = 3
_STAGE = 4   # 1=gather 2=+segmm 3=+Wmatmul+elu 4=full

ROWS_PER_CORE = (N_NODES + N_CORES - 1) // N_CORES          # 12500
TILES = (ROWS_PER_CORE + P - 1) // P                        # 98
ROWS_PAD = TILES * P                                        # 12544


def _host_prep(indices, values, features):
    """Sort edges by (core, tile, bank); build per-core gather-idx /
    dest-local / value arrays with a globally uniform group structure."""
    row = np.asarray(indices[0]).astype(np.int64)
    col = np.asarray(indices[1]).astype(np.int64)
    vals = np.asarray(values).astype(np.float32)
    n_banks = (N_NODES + BANK - 1) // BANK                   # 4

    core = row // ROWS_PER_CORE
    rloc = row % ROWS_PER_CORE
    t = rloc // P
    dl = rloc % P
    b = col // BANK
    ib = col % BANK

    order = np.lexsort((col, b, t, core))
    core, t, dl, b, ib, v = (core[order], t[order], dl[order], b[order],
                             ib[order], vals[order])

    # counts per (core, tile, bank)
    seg_id = (core * TILES + t) * n_banks + b
    n_segs = N_CORES * TILES * n_banks
    counts = np.bincount(seg_id, minlength=n_segs).reshape(N_CORES, TILES,
                                                           n_banks)
    # uniform groups per bank (same for every core/tile)
    G = np.maximum(1, ((counts.max(axis=(0, 1)) + P - 1) // P)).astype(int)
    G_tile = int(G.sum())                                    # groups per tile
    slots_tile = G_tile * P
    goff = np.concatenate(([0], np.cumsum(G[:-1]))) * P      # slot offset of bank
    total_slots = TILES * slots_tile

    # slot position of each edge: seg base + rank within segment
    seg_start = np.zeros(n_segs + 1, np.int64)
    np.cumsum(counts.ravel(), out=seg_start[1:])
    rank = np.arange(len(core)) - seg_start[seg_id]
    slot = t * slots_tile + goff[b] + rank                   # within-core slot

    idx_arr = np.zeros((N_CORES, total_slots), np.int16)     # pad -> row 0
    dl_arr = np.zeros((N_CORES, total_slots), np.float32)
    v_arr = np.zeros((N_CORES, total_slots), np.float32)
    idx_arr[core, slot] = ib.astype(np.int16)
    dl_arr[core, slot] = dl.astype(np.float32)
    v_arr[core, slot] = v.astype(np.float32)

    # gather-idx wrapped layout [128, total_slots/16]: within each per-tile
    # call the i-th index sits at (i % 16, call_col + i // 16), replicated to
    # all 8 16-partition groups.
    ic = idx_arr.reshape(N_CORES, TILES, G_tile * P // 16, 16)
    idx_w = np.zeros((N_CORES, 128, TILES * slots_tile // 16), np.int16)
    base = np.transpose(ic, (0, 3, 1, 2)).reshape(N_CORES, 16, -1)
    for g8 in range(8):
        idx_w[:, g8 * 16:(g8 + 1) * 16, :] = base

    # dl/v [128, n_groups_total]: slot (t, g, p) -> column t*G_tile + g, row p
    dl_w = np.transpose(dl_arr.reshape(N_CORES, TILES * G_tile, P), (0, 2, 1))
    v_w = np.transpose(v_arr.reshape(N_CORES, TILES * G_tile, P), (0, 2, 1))
    return (G.tolist(), idx_w, np.ascontiguousarray(dl_w),
            np.ascontiguousarray(v_w))


def _build_program(G, n_banks, bank_rows):
    """One SPMD Bass program (per-core work; identical across cores)."""
    G_tile = int(sum(G))
    slots_tile = G_tile * P
    idx_cols = TILES * slots_tile // 16
    ncols_dlv = TILES * G_tile

    nc = bacc.Bacc("TRN2", num_devices=N_CORES)
    d_table = nc.dram_tensor("table", [BANK * (n_banks - 1) + bank_rows[-1],
                                       DIN], F16, kind="ExternalInput")
    d_idx = nc.dram_tensor("gidx", [128, idx_cols], I16, kind="ExternalInput")
    d_dl = nc.dram_tensor("dl", [128, ncols_dlv], F32, kind="ExternalInput")
    d_v = nc.dram_tensor("val", [128, ncols_dlv], F32, kind="ExternalInput")
    d_iota = nc.dram_tensor("iota", [128, 128], F16, kind="ExternalInput")
    d_w = nc.dram_tensor("wmat", [DIN, DOUT], F32, kind="ExternalInput")
    d_bias = nc.dram_tensor("biasc", [128, 1], F32, kind="ExternalInput")
    d_gam = nc.dram_tensor("gamb", [128, 128], F32, kind="ExternalInput")
    d_bet = nc.dram_tensor("betb", [128, 128], F32, kind="ExternalInput")
    d_eye = nc.dram_tensor("eye", [128, 128], F32, kind="ExternalInput")
    d_out = nc.dram_tensor("out", [ROWS_PAD, DOUT], F32, kind="ExternalOutput")

    with tile.TileContext(nc) as tc:
        with (
            tc.tile_pool(name="const", bufs=1) as cpool,
            tc.tile_pool(name="gin", bufs=1) as gpool,
            tc.tile_pool(name="dst", bufs=_DST_BUFS) as dpool,
            tc.tile_pool(name="smat", bufs=4) as spool,
            tc.tile_pool(name="psA", bufs=2, space="PSUM") as psA,
            tc.tile_pool(name="psB", bufs=2, space="PSUM") as psB,
            tc.tile_pool(name="epi", bufs=3) as epool,
            tc.tile_pool(name="ln", bufs=4) as lpool,
        ):
            sb_idx = gpool.tile([128, idx_cols], I16)
            nc.sync.dma_start(sb_idx[:], d_idx[:])
            sb_dl = gpool.tile([128, ncols_dlv], F32)
            nc.sync.dma_start(sb_dl[:], d_dl[:])
            sb_v = gpool.tile([128, ncols_dlv], F32)
            nc.sync.dma_start(sb_v[:], d_v[:])
            sb_iota = cpool.tile([128, 128], F16)
            nc.sync.dma_start(sb_iota[:], d_iota[:])
            sb_w = cpool.tile([DIN, DOUT], F32)
            nc.sync.dma_start(sb_w[:], d_w[:])
            sb_bias = cpool.tile([128, 1], F32)
            nc.sync.dma_start(sb_bias[:], d_bias[:])
            sb_gam = cpool.tile([128, 128], F32)
            nc.sync.dma_start(sb_gam[:], d_gam[:])
            sb_bet = cpool.tile([128, 128], F32)
            nc.sync.dma_start(sb_bet[:], d_bet[:])
            sb_eye = cpool.tile([128, 128], F32)
            nc.sync.dma_start(sb_eye[:], d_eye[:])

            for t in range(TILES):
                # -- gather this tile's slots (one call per bank) --
                dst = dpool.tile([128, G_tile, DIN], F16, tag="dst")
                goff = 0
                icol = t * (slots_tile // 16)
                for b in range(n_banks):
                    ni = G[b] * P
                    nc.gpsimd.dma_gather(
                        dst[:, goff:goff + G[b], :],
                        d_table[b * BANK: b * BANK + bank_rows[b], :],
                        sb_idx[:, icol:icol + ni // 16],
                        ni, ni, DIN, single_packet=False,
                    )
                    goff += G[b]
                    icol += ni // 16

                if _STAGE == 1:
                    outt = epool.tile([128, 128], F16, tag="g1")
                    nc.vector.tensor_copy(outt[:], dst[:, 0, :])
                    yo32 = epool.tile([128, 128], F32, tag="g2")
                    nc.vector.tensor_copy(yo32[:], outt[:])
                    nc.sync.dma_start(d_out[t * P:(t + 1) * P, :], yo32[:])
                    continue
                # -- segment matmuls: psum[feat, dest] += Xg.T @ S --
                ps = psA.tile([128, 128], F32, tag="agg")
                for g in range(G_tile):
                    c = t * G_tile + g
                    s_t = spool.tile([128, 128], F16, tag="S")
                    nc.vector.tensor_scalar(
                        s_t[:], sb_iota[:], sb_dl[:, c:c + 1], sb_v[:, c:c + 1],
                        OP.is_equal, OP.mult)
                    nc.tensor.matmul(ps[:], dst[:, g, :], s_t[:],
                                     start=(g == 0), stop=(g == G_tile - 1))

                # -- epilogue --
                aggT = epool.tile([128, 128], F32, tag="aggT")
                nc.scalar.copy(aggT[:], ps[:])              # psum -> sbuf
                if _STAGE == 2:
                    nc.sync.dma_start(d_out[t * P:(t + 1) * P, :], aggT[:])
                    continue
                zps = psB.tile([128, 128], F32, tag="z")
                nc.tensor.matmul(zps[:], sb_w[:], aggT[:], start=True,
                                 stop=True)                 # [dout, nodes]
                z1 = epool.tile([128, 128], F32, tag="z1")
                nc.vector.tensor_scalar(z1[:], zps[:], sb_bias[:], None,
                                        OP.add)             # + bias (per feat)
                ex = epool.tile([128, 128], F32, tag="ex")
                nc.scalar.activation(ex[:], z1[:], ACT.Exp)
                e1 = epool.tile([128, 128], F32, tag="e1")
                nc.vector.tensor_scalar(e1[:], ex[:], 1.0, -1.0, OP.min,
                                        OP.add)             # min(e,1)-1
                rl = epool.tile([128, 128], F32, tag="rl")
                nc.scalar.activation(rl[:], z1[:], ACT.Relu)
                hT = epool.tile([128, 128], F32, tag="hT")
                nc.vector.tensor_tensor(hT[:], rl[:], e1[:], OP.add)
                if _STAGE == 3:
                    nc.sync.dma_start(d_out[t * P:(t + 1) * P, :], hT[:])
                    continue

                hps = psB.tile([128, 128], F32, tag="hps")
                nc.tensor.transpose(hps[:], hT[:], sb_eye[:])
                h = epool.tile([128, 128], F32, tag="h")
                nc.scalar.copy(h[:], hps[:])                # [nodes, feat]
                if _STAGE == 35:
                    nc.sync.dma_start(d_out[t * P:(t + 1) * P, :], h[:])
                    continue

                # LayerNorm over feature (free) dim
                s1 = lpool.tile([128, 1], F32, tag="s1")
                nc.vector.reduce_sum(s1[:], h[:], axis=AX.X)
                if _STAGE == 36:
                    nc.sync.dma_start(d_out[t * P:(t + 1) * P, :], h[:])
                    continue
                sq = epool.tile([128, 128], F32, tag="sq")
                nc.vector.tensor_tensor(sq[:], h[:], h[:], OP.mult)
                msq = lpool.tile([128, 1], F32, tag="msq")
                nc.vector.reduce_sum(msq[:], sq[:], axis=AX.X)
                nc.vector.tensor_scalar(msq[:], msq[:], 1.0 / 128, None,
                                        OP.mult)
                mu = lpool.tile([128, 1], F32, tag="mu")
                nc.vector.tensor_scalar(mu[:], s1[:], 1.0 / 128, None, OP.mult)
                if _STAGE == 37:
                    nc.sync.dma_start(d_out[t * P:(t + 1) * P, :], sq[:])
                    continue
                var = lpool.tile([128, 1], F32, tag="var")
                nc.vector.tensor_scalar(var[:], mu[:], mu[:], None, OP.mult)
                nc.vector.tensor_scalar(var[:], var[:], msq[:], -1.0,
                                        OP.subtract, OP.mult)  # msq - mu^2
                nc.vector.tensor_scalar(var[:], var[:], EPS, None, OP.add)
                std = lpool.tile([128, 1], F32, tag="std")
                nc.scalar.sqrt(std[:], var[:])
                rstd = lpool.tile([128, 1], F32, tag="rstd")
                nc.vector.reciprocal(rstd[:], std[:])
                if _STAGE == 39:
                    nc.sync.dma_start(d_out[t * P:(t + 1) * P, :], sq[:])
                    continue
                y = epool.tile([128, 128], F32, tag="y")
                nc.vector.tensor_scalar(y[:], h[:], mu[:], rstd[:],
                                        OP.subtract, OP.mult)
                yg = epool.tile([128, 128], F32, tag="yg")
                nc.vector.tensor_tensor(yg[:], y[:], sb_gam[:], OP.mult)
                yo = epool.tile([128, 128], F32, tag="yo")
                nc.vector.tensor_tensor(yo[:], yg[:], sb_bet[:], OP.add)
                nc.sync.dma_start(d_out[t * P:(t + 1) * P, :], yo[:])
    nc.compile()
    return nc


_CACHE = {}


def kernel(indices, values, features, weight, bias, gamma, beta):
    G, idx_w, dl_w, v_w = _host_prep(indices, values, features)
    n_banks = (N_NODES + BANK - 1) // BANK
    bank_rows = [min(BANK, N_NODES - b * BANK) for b in range(n_banks)]

    key = tuple(G)
    if key not in _CACHE:
        _CACHE[key] = _build_program(G, n_banks, bank_rows)
    nc = _CACHE[key]

    table = np.ascontiguousarray(np.asarray(features).astype(np.float16))
    w32 = np.asarray(weight).astype(np.float32)
    bias_col = np.asarray(bias).astype(np.float32).reshape(DOUT, 1)
    gam_b = np.tile(np.asarray(gamma).astype(np.float32).reshape(1, DOUT),
                    (P, 1))
    bet_b = np.tile(np.asarray(beta).astype(np.float32).reshape(1, DOUT),
                    (P, 1))
    iota = np.tile(np.arange(128, dtype=np.float16).reshape(1, 128), (128, 1))
    eye = np.eye(128, dtype=np.float32)

    in_maps = []
    for c in range(N_CORES):
        in_maps.append({
            "table": table, "gidx": idx_w[c], "dl": dl_w[c], "val": v_w[c],
            "iota": iota, "wmat": w32, "biasc": bias_col, "gamb": gam_b,
            "betb": bet_b, "eye": eye,
        })
    res = bass_utils.run_bass_kernel_spmd(nc, in_maps,
                                          core_ids=list(range(N_CORES)))
    out = np.concatenate(
        [res.results[c]["out"][:ROWS_PER_CORE] for c in range(N_CORES)],
        axis=0)[:N_NODES]
    return out.astype(np.float32)
